# revision 73
# baseline (speedup 1.0000x reference)
"""Trainium2 Bass kernel for nn_Discriminator (down-projection + 16-step LSTM + head).

Computation (per reference):
    x: [512, 16, 10001] fp32
    xa = x[:, :, :10000] @ W_down                      # [B, T, 128]
    xc_t = concat([xa_t, xw_t], -1)                    # per step, [B, 129]
    LSTM over T=16 steps, H=512, forget bias +1:
        gates = [xc_t, h] @ W_cell + b_cell            # [B, 2048] = [i|c|f|o]
        c = c*sig(f+1) + sig(i)*tanh(c_)
        h = sig(o)*tanh(c_new)
    pred = h @ W_out + b_out                           # [B, 1]

Sharding: pure data-parallel over batch, 64 rows/core on 8 cores. No collectives.

Per-core design (measured ~147-150 us; the previous baseline was ~150-155 us
with a ~36 us dead DMA head and ~6.3-6.6 us steady step period):
  - Everything is fp16 (not bf16): same DMA bytes and PE rate, 8x the
    mantissa, which roughly halves the final error (1.1e-2 vs 2e-2 gate).
  - x is pre-transposed/tiled on the host so the PE contraction dim (n) is
    on partitions and every down-projection chunk DMA is one contiguous
    flat run per partition.
  - The DMA rings are overhead-bound at small transfer sizes (~600-1100 ns
    serialized per dma_start; one ring saturates at ~160 GB/s with 327 KB
    pieces but the 16 shared DMA engines reach ~360 GB/s aggregate with MB-
    sized transfers), so x moves in 1.3-2 MB dma_start pieces, ALL on the
    sync ring, in strict priority order: W_down -> ch0 -> Wxa -> Wh0 ->
    ch1a -> Wh1 -> ch1b -> ch2..., so x chunks can't starve the sigma0/1-
    critical weights. The scalar ring carries only small tensors, keeping
    the ACT sequencer free of DMA work. Chunk SBUF buffers recycle through
    a 3-deep pool; the WAR dependency on the pool buffer self-paces the
    ring, and chunks 3+ are emitted lazily after chunk c-3's last read.
  - Chunk widths [128, 128, 192x4] (chunk c covers 2-3 steps): wider
    streams amortize the per-k-tile W_down reload (load-bound at 128 cols);
    the first chunk stays narrow so sigma0 fires at ~30 us.
  - The xw (walk-weight) + bias gate contribution is a rank-1 update
    (xw[b,t]*w[g] + b[g]); it is computed on the ACT engine as a per-step
    "seed" of the gate PSUM pair: activation(Identity, in=Wbc quadrant
    tile, scale=xw2[:,t] per-partition, bias=bq per-partition). All gate
    matmuls then accumulate onto the seed with start=False
    (skip_group_check: the sim's zero-region bookkeeping misaligns for
    partition-offset quadrants and walrus miscompiles without it). This
    removes ~0.9 us/step of tiny-lhsT PE matmuls.
  - LSTM gates for a step live in TWO single-bank PSUM tiles g0/g1 (one per
    512-col gate bank) so the bank-0 sigmoid's dependency releases as soon
    as bank 0's matmuls finish, overlapping bank 1's matmul stream:
      g0: p0:64 = c~, p64:128 = f;  g1: p0:64 = i, p64:128 = o
    The two half-quadrant matmuls of a (bank, k) share their stationary
    hT k-tile and are emitted adjacently — the PE then runs the second
    stream nearly free (observed ~3 ns + 213 ns pairs).
  - All four gates use a single sigmoid: tanh is folded into sigmoid on the
    host (tanh(x) = 2*sig(2x)-1, c-gate columns pre-scaled by 2), the
    forget bias +1 is folded into b_cell (rides the seed). (A variant using
    the ACT Tanh table directly for c~ measured SLOWER: the extra ACT op
    delays sigma_b behind the seed activations in the 4-deep wait queue.)
  - The cell update uses scalar_tensor_tensor fusions; c_new's tanh uses
    the ACT Tanh function (same table set as Sigmoid, so no reload). The
    c/h tail is split into two 256-column halves so stt/Tanh/h/transpose
    pipeline across engines. m1 stays in fp32 PSUM (SBUF operands of one
    DVE op must share a base partition; the DVE cannot write 16-bit PSUM).
  - h (fp16, partitions 64:128) is PE-transposed (4x [64,128]) into one
    PSUM tile, then copied per k-tile so the next step's bank-0 matmul for
    k-tile k only waits on copy k.
  - W_h is stored bank-major so the bank-0 half can be DMA'd ahead of the
    bank-1 half (step 1's first sigmoid needs only bank 0).
  - Chunk c's dp matmuls are emitted in the activation-chain shadows of the
    2-3 steps before its first use (k-ranges matching its DMA pieces, so a
    partially-arrived chunk can start), with the PSUM->SBUF copy at the end
    of the last emission step, right before the xa prefetch that reads it.
    Warm-keeper matmuls keep the PE's HAM clock gate high once the dp
    queue drains (steps 13-14).
"""

import numpy as np
from contextlib import ExitStack

NCORES = 8
B = 512
BC = B // NCORES          # 64 batch rows per core
T = 16
BT = BC * T               # 1024
N = 10000
KT = 79                   # ceil(10000/128)
NPAD = KT * 128           # 10112
HIN = 128
H = 512
G4 = 4 * H                # 2048
# down-projection output chunks (col-start, width) in xa^T columns (t*64+b,
# t-major). The first chunk covers steps 0-1; later chunks widen (wider PE
# streams amortize the per-k-tile weight reload).
CHUNKS = [(0, 128), (128, 128), (256, 192), (448, 192), (640, 192), (832, 192)]
NCH = len(CHUNKS)
KSPLIT = 40               # k split for the two W_down DMA pieces

# fp16 over bf16: same DMA bytes and PE rate, 8x the mantissa precision, and
# 16-bit chain tensors keep the DVE in 2x packed mode
MM_DTYPE = "float16"

_CACHE = {}


def _np_mm_dtype():
    if MM_DTYPE == "bfloat16":
        import ml_dtypes
        return ml_dtypes.bfloat16
    if MM_DTYPE == "float16":
        return np.float16
    return np.float32


def _first_step(c):
    # first LSTM step that reads chunk c
    return c if c < 2 else 2 * c - 2


def _build_module():
    import concourse.bass as bass  # noqa: F401
    import concourse.bacc as bacc
    import concourse.tile as tile
    import concourse.mybir as mybir

    AF = mybir.ActivationFunctionType
    ALU = mybir.AluOpType
    f32 = mybir.dt.float32
    mmdt = {"bfloat16": mybir.dt.bfloat16, "float16": mybir.dt.float16,
            "float32r": mybir.dt.float32r}[MM_DTYPE]

    nc = bacc.Bacc("TRN2")

    # x pre-tiled on host per chunk to [128, KT, width] (flattened along the
    # free dim) so every chunk DMA is one contiguous run per partition
    xT = nc.declare_dram_parameter("xT", [128, KT * BT], mmdt, isOutput=False)
    # xw2[p, t] = x_weights[p mod 64, t]; Wbc/b0 are the xw-row of W_cell and
    # gate bias folded to the gate-psum quadrant layout (per-partition)
    xw2 = nc.declare_dram_parameter("xw2", [128, T], f32, isOutput=False)
    Wbc = nc.declare_dram_parameter("Wbc", [128, 2 * H], mmdt, isOutput=False)
    bq = nc.declare_dram_parameter("bq", [128, 2], f32, isOutput=False)
    Wd = nc.declare_dram_parameter("Wd", [128, KT * 128], mmdt, isOutput=False)
    Wxa = nc.declare_dram_parameter("Wxa", [128, G4], mmdt, isOutput=False)
    # bank-major: col = bank*4096 + k*1024 + half*512 + j
    Wh = nc.declare_dram_parameter("Wh", [128, 4 * G4], mmdt, isOutput=False)
    Wo = nc.declare_dram_parameter("Wo", [128, H], mmdt, isOutput=False)
    bout = nc.declare_dram_parameter("bout", [128, 1], f32, isOutput=False)
    ident = nc.declare_dram_parameter("ident", [128, BC], mmdt, isOutput=False)
    pred = nc.declare_dram_parameter("pred", [BC, 1], f32, isOutput=True)

    # flat free-dim offset of each chunk in xT
    choff = [0]
    for _, w in CHUNKS:
        choff.append(choff[-1] + KT * w)

    with ExitStack() as ctx:
        tc = ctx.enter_context(tile.TileContext(nc))
        singles = ctx.enter_context(tc.tile_pool(name="singles", bufs=1))
        xch = ctx.enter_context(tc.tile_pool(name="xch", bufs=3))
        work = ctx.enter_context(tc.tile_pool(name="work", bufs=2))
        state = ctx.enter_context(tc.tile_pool(name="state", bufs=2))
        dpp = ctx.enter_context(tc.tile_pool(name="dpp", bufs=2, space="PSUM"))
        gp = ctx.enter_context(tc.tile_pool(name="gp", bufs=4, space="PSUM"))
        tp = ctx.enter_context(tc.tile_pool(name="tp", bufs=2, space="PSUM"))

        # ---- small tensors first on the scalar ring (seed(0) needs them) ----
        xw2_sb = singles.tile([128, T], f32)
        nc.scalar.dma_start(xw2_sb[:], xw2[:])
        Wbc_sb = singles.tile([128, 2 * H], mmdt)
        nc.scalar.dma_start(Wbc_sb[:], Wbc[:])
        bq_sb = singles.tile([128, 2], f32)
        nc.scalar.dma_start(bq_sb[:], bq[:])

        # dummy activation so the sigmoid-set ACT_TABLE_LOAD (~1.3 us) runs
        # here, hidden under the DMA-paced head
        warmup = singles.tile([1, 1], f32)
        nc.vector.memset(warmup[:], 0.0)
        nc.scalar.activation(warmup[:], warmup[:], AF.Sigmoid)

        # ---- big loads ----
        # Everything sigma0/sigma1-critical rides the SYNC ring in strict
        # priority order (Wd -> ch0 -> Wxa -> Wh) so x chunks can't starve
        # the weights; later chunks follow. The scalar ring only carries the
        # small tensors, keeping the ACT sequencer free of DMA work.
        cht = [None] * NCH
        # k-piece boundaries per chunk (also the SCHED emission splits)
        KSPLITS = {0: [0, 40, KT], 1: [0, 40, KT], 2: [0, 40, KT],
                   3: [0, 26, 52, KT], 4: [0, 26, 52, KT], 5: [0, 26, 52, KT]}

        def chunk_dma(c, piece=None):
            w = CHUNKS[c][1]
            if cht[c] is None:
                cht[c] = xch.tile([128, KT * 192], mmdt, tag="xch",
                                  name=f"xch{c}")
            t_ = cht[c]
            ks = KSPLITS[c]
            pieces = list(zip(ks[:-1], ks[1:]))
            if piece is not None:
                pieces = [pieces[piece]]
            for k0, k1 in pieces:
                nc.sync.dma_start(t_[:, k0 * w:k1 * w],
                                  xT[:, choff[c] + k0 * w:choff[c] + k1 * w])

        Wd_sb = singles.tile([128, KT * 128], mmdt)
        nc.sync.dma_start(Wd_sb[:, :KSPLIT * 128], Wd[:, :KSPLIT * 128])
        chunk_dma(0, piece=0)
        nc.sync.dma_start(Wd_sb[:, KSPLIT * 128:], Wd[:, KSPLIT * 128:])
        chunk_dma(0, piece=1)
        Wxa_sb = singles.tile([128, G4], mmdt)
        nc.sync.dma_start(Wxa_sb[:], Wxa[:])
        Wh_sb = singles.tile([128, 4 * G4], mmdt)
        nc.sync.dma_start(Wh_sb[:, :2 * G4], Wh[:, :2 * G4])         # bank 0
        chunk_dma(1, piece=0)
        nc.sync.dma_start(Wh_sb[:, 2 * G4:], Wh[:, 2 * G4:])         # bank 1
        chunk_dma(1, piece=1)
        # chunk 2 claims the third pool buffer now; chunks 3+ are emitted
        # lazily after chunk c-3's last dp matmul (the pool WAR dependency
        # is trace-order-based, so the DMA must be emitted after the reads
        # it might overwrite)
        chunk_dma(2)

        id_sb = singles.tile([128, BC], mmdt)
        nc.scalar.dma_start(id_sb[:], ident[:])
        Wo_sb = singles.tile([128, H], mmdt)
        nc.scalar.dma_start(Wo_sb[:], Wo[:])
        bout_sb = singles.tile([128, 1], f32)
        nc.scalar.dma_start(bout_sb[:], bout[:])

        # quadrant (bank, half) of permuted gates = cols half*1024+bank*512
        def rh_quad(rh, bank, half):
            return rh[:, half * 2 * H + bank * H: half * 2 * H + (bank + 1) * H]

        # ---- down-projection machinery ----
        xaT_sb = [singles.tile([128, w], mmdt, name=f"xaT{c}")
                  for c, (_, w) in enumerate(CHUNKS)]

        def dp_mms(c, ps, k0, k1):
            w = CHUNKS[c][1]
            t_ = cht[c]
            for k in range(k0, k1):
                nc.tensor.matmul(ps[:], Wd_sb[:, k * 128:(k + 1) * 128],
                                 t_[:, k * w:(k + 1) * w],
                                 start=(k == 0), stop=(k == KT - 1))

        def chunk_copy(c, ps):
            # PSUM -> SBUF on the DVE (gpsimd cannot access PSUM); emitted in
            # post_chain so it sits behind the step's chain ops in the FIFO
            nc.vector.tensor_copy(xaT_sb[c][:], ps[:])

        # chunk 0: full dp in the head (DMA-paced)
        cur_ps = [None] * NCH
        cur_ps[0] = dpp.tile([128, 512], mybir.dt.float32, tag="dp",
                             name="dps0")[:, :CHUNKS[0][1]]
        dp_mms(0, cur_ps[0], 0, KT)
        chunk_dma(3)
        chunk_copy(0, cur_ps[0])

        # ---- LSTM ----
        # All chain tensors (incl. the cell state c) are 16-bit so every DVE
        # op runs in 2x packed mode; fp16's 10 mantissa bits keep the c
        # accumulation error ~2^-11 per step.
        hT_prev = None
        U_cur = work.tile([128, H], mmdt, tag="u")
        nc.vector.memset(U_cur[64:128, :], 0.0)
        g_next = None

        def seed(t):
            """Allocate the psum pair for step t and seed it with the xw/bias
            gate contribution on the ACT engine (out = Wbc*xw[t] + b, a rank-1
            update the PE would otherwise burn ~0.9 us/step on). Emitted at
            the TOP of the previous lstm_step so the seeds run in the ACT's
            idle window there (the 4-deep wait queue otherwise lets them
            slip between sigma_a and sigma_b, delaying the chain)."""
            g0 = gp.tile([128, H], mybir.dt.float32, tag="g", name="g0")
            g1 = gp.tile([128, H], mybir.dt.float32, tag="g", name="g1")
            nc.scalar.activation(g0[:], Wbc_sb[:, 0:H], AF.Identity,
                                 bias=bq_sb[:, 0:1], scale=xw2_sb[:, t:t + 1])
            nc.scalar.activation(g1[:], Wbc_sb[:, H:2 * H], AF.Identity,
                                 bias=bq_sb[:, 1:2], scale=xw2_sb[:, t:t + 1])
            return (g0, g1)

        def xa_mms(t, gpair, close=False):
            """Accumulate step t's xa gate contributions onto its seeded
            psum pair (start=False)."""
            col = t * BC
            ci = next(i for i, (o, w) in enumerate(CHUNKS) if o <= col < o + w)
            coff = col - CHUNKS[ci][0]
            lh = xaT_sb[ci][:, coff:coff + BC]
            for bank, g in ((0, gpair[0]), (1, gpair[1])):
                for half in range(2):
                    # skip_group_check: the sim's psum zero-region bookkeeping
                    # misaligns for partition-offset quadrants (HW zeroes only
                    # the written bytes); the seed supplies the initial value
                    nc.tensor.matmul(g[half * 64:(half + 1) * 64, :], lh,
                                     rh_quad(Wxa_sb, bank, half),
                                     start=False, stop=close,
                                     skip_group_check=True)

        def lstm_step(t, pe_fill=None, post_chain=None, pe_fill_post=None):
            nonlocal hT_prev, U_cur, g_next
            g0, g1 = g_next
            Sa = work.tile([128, H], mmdt, tag="sa")
            Sb = work.tile([128, H], mmdt, tag="sb")

            def acts_a():
                nc.scalar.activation(Sa[:], g0[:], AF.Sigmoid)

            if hT_prev is not None:
                for bank, g in ((0, g0), (1, g1)):
                    for k in range(4):
                        lh = hT_prev[:, k, :]
                        sp = k == 3
                        for half in range(2):
                            rhap = Wh_sb[:, bank * 4 * G4 // 2 + k * 2 * H
                                         + half * H: bank * 4 * G4 // 2
                                         + k * 2 * H + (half + 1) * H]
                            nc.tensor.matmul(g[half * 64:(half + 1) * 64, :],
                                             lh, rhap, start=False, stop=sp,
                                             skip_group_check=True)
                    if bank == 0:
                        acts_a()
                nc.scalar.activation(Sb[:], g1[:], AF.Sigmoid)
            else:
                acts_a()
                nc.scalar.activation(Sb[:], g1[:], AF.Sigmoid)

            # dp slab matmuls in the activation-chain shadow
            if pe_fill is not None:
                pe_fill()

            # Gate layout after the host fold (tanh(x) = 2*sig(2x)-1 with
            # c-gate columns pre-scaled by 2; forget bias +1 folded in):
            #   Sa[0:64] = sig(2*c~)   Sa[64:128] = sig(f+1)
            #   Sb[0:64] = sig(i)      Sb[64:128] = sig(o)
            #   m1 (psum, p64:128) = sig(f+1)*c_prev
            #   m2 (sbuf, p0:64)   = (sig(2c~)-0.5)*sig(i) = tanh(c~)*sig(i)/2
            # m1 lives in PSUM (SBUF operands of one DVE op must share a base
            # partition, and m2 sits at partitions 0:64 while m1 is at
            # 64:128) and must be fp32 (the DVE cannot write 16-bit PSUM)
            m1 = tp.tile([128, H], mybir.dt.float32, tag="tp", name="m1")
            nc.vector.tensor_mul(m1[64:128, :], Sa[64:128, :], U_cur[64:128, :])
            HH = H // 2
            m2 = work.tile([BC, H], mmdt, tag="m2")
            for x0 in (0, HH):
                nc.vector.scalar_tensor_tensor(m2[:, x0:x0 + HH], Sa[0:64, x0:x0 + HH],
                                               0.5, Sb[0:64, x0:x0 + HH],
                                               ALU.subtract, ALU.mult)
            U_nxt = work.tile([128, H], mmdt, tag="u")
            scn = work.tile([128, H], mmdt, tag="scn")
            for x0 in (0, HH):
                nc.vector.scalar_tensor_tensor(U_nxt[64:128, x0:x0 + HH], m2[:, x0:x0 + HH],
                                               2.0, m1[64:128, x0:x0 + HH],
                                               ALU.mult, ALU.add)
                nc.scalar.activation(scn[64:128, x0:x0 + HH], U_nxt[64:128, x0:x0 + HH],
                                     AF.Tanh)

            h = work.tile([128, H], mmdt, tag="h")
            hT = state.tile([128, 4, BC], mmdt, tag="hT")
            tps = tp.tile([128, 4 * BC], mmdt, tag="tp", name="tps")
            for half in range(2):
                nc.vector.tensor_mul(h[64:128, half * HH:(half + 1) * HH],
                                     scn[64:128, half * HH:(half + 1) * HH],
                                     Sb[64:128, half * HH:(half + 1) * HH])
                if t + 1 >= T:
                    continue          # pred reads h directly, not hT
                for k in (2 * half, 2 * half + 1):
                    nc.tensor.transpose(tps[:, k * BC:(k + 1) * BC],
                                        h[64:128, k * 128:(k + 1) * 128], id_sb[64:128, :])
                for k in (2 * half, 2 * half + 1):
                    # ACT (idle at the chain tail) instead of the DVE, whose
                    # queue still holds the h-muls; Copy is in the loaded set
                    nc.scalar.activation(hT[:, k, :], tps[:, k * BC:(k + 1) * BC],
                                         AF.Copy)
            # chunk-completion copies, then the next step's seed + xa
            # prefetch (which reads the copied tile)
            if post_chain is not None:
                post_chain()
            if t + 1 < T:
                g_next = seed(t + 1)
                xa_mms(t + 1, g_next)
            if pe_fill_post is not None:
                pe_fill_post()
            hT_prev = hT
            U_cur = U_nxt
            return h

        g_next = seed(0)
        xa_mms(0, g_next, close=True)
        pending_copies = []

        # chunk c>=1: matmuls spread over the steps before its first use
        # (k-ranges match the chunk's DMA pieces so each range only waits on
        # its own piece), copy at the end of the last emission step
        SCHED = {}
        _WINDOWS = {1: [0, 1], 2: [2, 3], 3: [4, 5, 6], 4: [7, 8, 9],
                    5: [10, 11, 12]}
        for c, steps in _WINDOWS.items():
            ks = KSPLITS[c]
            for s, k0, k1 in zip(steps, ks[:-1], ks[1:]):
                SCHED.setdefault(s, []).append((c, k0, k1))
        filled = {}

        def dp_fill(t):
            n = 0
            for c, k0, k1 in SCHED.get(t, []):
                if k0 == 0:
                    cur_ps[c] = dpp.tile([128, 512], mybir.dt.float32,
                                         tag="dp", name=f"dps{c}")[:, :CHUNKS[c][1]]
                dp_mms(c, cur_ps[c], k0, k1)
                if k1 == KT:
                    if c + 3 < NCH:
                        chunk_dma(c + 3)
                    pending_copies.append(c)
                n += 1
            filled[t] = n

        def tail_fill():
            while pending_copies:
                c = pending_copies.pop(0)
                chunk_copy(c, cur_ps[c])

        def make_warm(t):
            def warm_fill():
                if filled.get(t, 0) == 0 and t + 1 < T and hT_prev is not None:
                    warm = dpp.tile([128, 512], mybir.dt.float32, tag="dp", name="warm")
                    NW = 9
                    for w in range(NW):
                        nc.tensor.matmul(warm[0:64, :], hT_prev[:, w % 4, :],
                                         Wh_sb[:, 0:512], start=(w == 0), stop=(w == NW - 1))
            return warm_fill

        for t in range(T):
            h_last = lstm_step(t, pe_fill=(lambda t=t: dp_fill(t)),
                               post_chain=tail_fill, pe_fill_post=make_warm(t))

        # output head: pred = reduce(h * W_out_row) + b_out on the DVE --
        # no hT transposes/copies or PE matmuls on the tail
        scr = singles.tile([128, H], mmdt)
        red = singles.tile([128, 1], mybir.dt.float32)
        out_sb = singles.tile([128, 1], mybir.dt.float32)
        nc.vector.tensor_mul(scr[64:128, :], h_last[64:128, :], Wo_sb[64:128, :])
        nc.vector.tensor_reduce(red[64:128, :], scr[64:128, :],
                                mybir.AxisListType.X, ALU.add)
        nc.vector.tensor_add(out_sb[64:128, :], red[64:128, :], bout_sb[64:128, :])
        nc.sync.dma_start(pred[:], out_sb[64:128, :])

    nc.finalize()
    return nc


def _get_module():
    key = MM_DTYPE
    if key not in _CACHE:
        _CACHE[key] = _build_module()
    return _CACHE[key]


def _prep_inputs(x, W_down, W_cell, b_cell, W_out, b_out):
    mmnp = _np_mm_dtype()
    x = np.asarray(x, dtype=np.float32)
    W_down = np.asarray(W_down, dtype=np.float32)
    W_cell = np.asarray(W_cell, dtype=np.float32)
    b_cell = np.asarray(b_cell, dtype=np.float32)
    W_out = np.asarray(W_out, dtype=np.float32)
    b_out = np.asarray(b_out, dtype=np.float32)

    # shared tensors
    Wd_pad = np.zeros((NPAD, HIN), dtype=np.float32)
    Wd_pad[:N] = W_down
    # [NPAD, 128] -> per-k-tile layout [128, KT*128] (col block k = k-tile)
    Wd_host = np.ascontiguousarray(
        Wd_pad.reshape(KT, 128, HIN).transpose(1, 0, 2).reshape(128, KT * HIN)
    ).astype(mmnp)
    # Fold the LSTM's fixed affine pieces into the weights so the device can
    # use a single sigmoid for the gate banks (tanh(x) = 2*sig(2x) - 1 for
    # the c~ gate; c_new's tanh uses the ACT Tanh function directly):
    #   - c-gate columns (512:1024) scaled by 2  -> sig computes sig(2*c~)
    #   - forget bias +1 folded into b_cell
    Wmod = W_cell.astype(np.float64).copy()
    b_mod = b_cell.astype(np.float64).copy()
    b_mod[1024:1536] += 1.0
    Wmod[:, 512:1024] *= 2.0
    b_mod[512:1024] *= 2.0
    # permute gate columns [i|c|f|o] -> [c~|i|f|o]: the device wants the
    # early-needed gates (c~, f) in PSUM bank 0 of each partition half
    perm = np.concatenate([np.arange(512, 1024), np.arange(0, 512),
                           np.arange(1024, 1536), np.arange(1536, 2048)])
    Wmod = Wmod[:, perm]
    b_mod = b_mod[perm]
    Wxa_host = np.ascontiguousarray(Wmod[0:HIN]).astype(mmnp)            # [128, 2048]
    # xw-row of W_cell and gate bias in gate-psum quadrant layout:
    # quadrant (bank, half) = permuted cols half*1024+bank*512; tile col j of
    # bank b holds w[quad(b, p<64 ? 0 : 1)][j] on partition p
    w_row = Wmod[HIN]                                                    # [2048]
    Wbc_host = np.empty((128, 2 * H), dtype=np.float64)
    bq_host = np.empty((128, 2), dtype=np.float64)
    for bank in range(2):
        for half in range(2):
            quad = slice(half * 1024 + bank * 512, half * 1024 + (bank + 1) * 512)
            Wbc_host[half * 64:(half + 1) * 64, bank * H:(bank + 1) * H] = w_row[quad]
            bvals = b_mod[quad]
            assert np.ptp(bvals) == 0.0, "gate bias must be quadrant-constant"
            bq_host[half * 64:(half + 1) * 64, bank] = bvals[0]
    Wbc_host = Wbc_host.astype(mmnp)
    bq_host = bq_host.astype(np.float32)
    # W_h bank-major: col = bank*4096 + k*1024 + half*512 + j
    Whm = Wmod[HIN + 1:]                                                 # [512, 2048]
    Whb = np.zeros((128, 4 * G4), dtype=np.float64)
    for bank in range(2):
        for k in range(4):
            for half in range(2):
                src = Whm[128 * k:128 * (k + 1),
                          half * 1024 + bank * 512: half * 1024 + (bank + 1) * 512]
                Whb[:, bank * 4096 + k * 1024 + half * 512:
                    bank * 4096 + k * 1024 + (half + 1) * 512] = src
    Wh_host = np.ascontiguousarray(Whb).astype(mmnp)                     # [128, 8192]
    Wo_host = np.ascontiguousarray(
        np.broadcast_to(W_out.reshape(1, H), (128, H))).astype(mmnp)     # [128, 512]
    bout_host = np.full((128, 1), float(b_out[0]), dtype=np.float32)
    id_host = np.concatenate([np.eye(BC, dtype=np.float32)] * 2, axis=0).astype(mmnp)

    in_maps = []
    for i in range(NCORES):
        xs = x[i * BC:(i + 1) * BC]                       # [64, 16, 10001]
        # xT: [NPAD, 1024], column index = t*64 + b (t-major)
        xT_host = np.zeros((NPAD, BT), dtype=mmnp)
        xT_host[:N] = xs[:, :, :N].transpose(2, 1, 0).reshape(N, BT).astype(mmnp)
        # re-tile per chunk to [128, KT, width] flattened on the free dim so
        # each (chunk, k-range) transfer is contiguous per partition:
        # chunk block[p, k, j] = xT[k*128 + p, off + j]
        parts = []
        for off, w in CHUNKS:
            blk = xT_host[:, off:off + w].reshape(KT, 128, w).transpose(1, 0, 2)
            parts.append(blk.reshape(128, KT * w))
        xT_host = np.ascontiguousarray(np.concatenate(parts, axis=1))
        xw2_host = np.concatenate([xs[:, :, N]] * 2, axis=0).astype(np.float32)
        in_maps.append({
            "xT": xT_host,
            "xw2": xw2_host,
            "Wbc": Wbc_host,
            "bq": bq_host,
            "Wd": Wd_host,
            "Wxa": Wxa_host,
            "Wh": Wh_host,
            "Wo": Wo_host,
            "bout": bout_host,
            "ident": id_host,
        })
    return in_maps


def run(trace=False, **inputs):
    from concourse.bass_utils import run_bass_kernel_spmd

    nc = _get_module()
    in_maps = _prep_inputs(**inputs)
    res = run_bass_kernel_spmd(nc, in_maps, list(range(NCORES)), trace=trace)
    pred = np.concatenate([res.results[i]["pred"] for i in range(NCORES)], axis=0)
    return pred.astype(np.float32), res


def kernel(**inputs):
    pred, _ = run(trace=False, **inputs)
    return pred


# revision 74
# speedup vs baseline: 1.0262x; 1.0262x over previous
"""Trainium2 Bass kernel for nn_Discriminator (down-projection + 16-step LSTM + head).

Computation (per reference):
    x: [512, 16, 10001] fp32
    xa = x[:, :, :10000] @ W_down                      # [B, T, 128]
    xc_t = concat([xa_t, xw_t], -1)                    # per step, [B, 129]
    LSTM over T=16 steps, H=512, forget bias +1:
        gates = [xc_t, h] @ W_cell + b_cell            # [B, 2048] = [i|c|f|o]
        c = c*sig(f+1) + sig(i)*tanh(c_)
        h = sig(o)*tanh(c_new)
    pred = h @ W_out + b_out                           # [B, 1]

Sharding: pure data-parallel over batch, 64 rows/core on 8 cores. No collectives.

Per-core design (measured ~147-150 us; the previous baseline was ~150-155 us
with a ~36 us dead DMA head and ~6.3-6.6 us steady step period):
  - Everything is fp16 (not bf16): same DMA bytes and PE rate, 8x the
    mantissa, which roughly halves the final error (1.1e-2 vs 2e-2 gate).
  - x is pre-transposed/tiled on the host so the PE contraction dim (n) is
    on partitions and every down-projection chunk DMA is one contiguous
    flat run per partition.
  - The DMA rings are overhead-bound at small transfer sizes (~600-1100 ns
    serialized per dma_start; one ring saturates at ~160 GB/s with 327 KB
    pieces but the 16 shared DMA engines reach ~360 GB/s aggregate with MB-
    sized transfers), so x moves in 1.3-2 MB dma_start pieces, ALL on the
    sync ring, in strict priority order: W_down -> ch0 -> Wxa -> Wh0 ->
    ch1a -> Wh1 -> ch1b -> ch2..., so x chunks can't starve the sigma0/1-
    critical weights. The scalar ring carries only small tensors, keeping
    the ACT sequencer free of DMA work. Chunk SBUF buffers recycle through
    a 3-deep pool; the WAR dependency on the pool buffer self-paces the
    ring, and chunks 3+ are emitted lazily after chunk c-3's last read.
  - Chunk widths [128, 128, 192x4] (chunk c covers 2-3 steps): wider
    streams amortize the per-k-tile W_down reload (load-bound at 128 cols);
    the first chunk stays narrow so sigma0 fires at ~30 us.
  - The xw (walk-weight) + bias gate contribution is a rank-1 update
    (xw[b,t]*w[g] + b[g]); it is computed on the ACT engine as a per-step
    "seed" of the gate PSUM pair: activation(Identity, in=Wbc quadrant
    tile, scale=xw2[:,t] per-partition, bias=bq per-partition). All gate
    matmuls then accumulate onto the seed with start=False
    (skip_group_check: the sim's zero-region bookkeeping misaligns for
    partition-offset quadrants and walrus miscompiles without it). This
    removes ~0.9 us/step of tiny-lhsT PE matmuls.
  - LSTM gates for a step live in TWO single-bank PSUM tiles g0/g1 (one per
    512-col gate bank) so the bank-0 sigmoid's dependency releases as soon
    as bank 0's matmuls finish, overlapping bank 1's matmul stream:
      g0: p0:64 = c~, p64:128 = f;  g1: p0:64 = i, p64:128 = o
    The two half-quadrant matmuls of a (bank, k) share their stationary
    hT k-tile and are emitted adjacently — the PE then runs the second
    stream nearly free (observed ~3 ns + 213 ns pairs).
  - All four gates use a single sigmoid: tanh is folded into sigmoid on the
    host (tanh(x) = 2*sig(2x)-1, c-gate columns pre-scaled by 2), the
    forget bias +1 is folded into b_cell (rides the seed). (A variant using
    the ACT Tanh table directly for c~ measured SLOWER: the extra ACT op
    delays sigma_b behind the seed activations in the 4-deep wait queue.)
  - The cell update uses scalar_tensor_tensor fusions; c_new's tanh uses
    the ACT Tanh function (same table set as Sigmoid, so no reload). The
    c/h tail is split into two 256-column halves so stt/Tanh/h/transpose
    pipeline across engines. m1 stays in fp32 PSUM (SBUF operands of one
    DVE op must share a base partition; the DVE cannot write 16-bit PSUM).
  - h (fp16, partitions 64:128) is PE-transposed (4x [64,128]) into one
    PSUM tile, then copied per k-tile so the next step's bank-0 matmul for
    k-tile k only waits on copy k.
  - W_h is stored bank-major so the bank-0 half can be DMA'd ahead of the
    bank-1 half (step 1's first sigmoid needs only bank 0).
  - Chunk c's dp matmuls are emitted in the activation-chain shadows of the
    2-3 steps before its first use (k-ranges matching its DMA pieces, so a
    partially-arrived chunk can start), with the PSUM->SBUF copy at the end
    of the last emission step, right before the xa prefetch that reads it.
    Warm-keeper matmuls keep the PE's HAM clock gate high once the dp
    queue drains (steps 13-14).
"""

import numpy as np
from contextlib import ExitStack

NCORES = 8
B = 512
BC = B // NCORES          # 64 batch rows per core
T = 16
BT = BC * T               # 1024
N = 10000
KT = 79                   # ceil(10000/128)
NPAD = KT * 128           # 10112
HIN = 128
H = 512
G4 = 4 * H                # 2048
# down-projection output chunks (col-start, width) in xa^T columns (t*64+b,
# t-major). The first chunk covers steps 0-1; later chunks widen (wider PE
# streams amortize the per-k-tile weight reload).
CHUNKS = [(0, 128), (128, 128), (256, 192), (448, 192), (640, 192), (832, 192)]
NCH = len(CHUNKS)
KSPLIT = 40               # k split for the two W_down DMA pieces

# fp16 over bf16: same DMA bytes and PE rate, 8x the mantissa precision, and
# 16-bit chain tensors keep the DVE in 2x packed mode
MM_DTYPE = "float16"

_CACHE = {}


def _np_mm_dtype():
    if MM_DTYPE == "bfloat16":
        import ml_dtypes
        return ml_dtypes.bfloat16
    if MM_DTYPE == "float16":
        return np.float16
    return np.float32


def _first_step(c):
    # first LSTM step that reads chunk c
    return c if c < 2 else 2 * c - 2


def _build_module():
    import concourse.bass as bass  # noqa: F401
    import concourse.bacc as bacc
    import concourse.tile as tile
    import concourse.mybir as mybir

    AF = mybir.ActivationFunctionType
    ALU = mybir.AluOpType
    f32 = mybir.dt.float32
    mmdt = {"bfloat16": mybir.dt.bfloat16, "float16": mybir.dt.float16,
            "float32r": mybir.dt.float32r}[MM_DTYPE]

    nc = bacc.Bacc("TRN2")

    # x pre-tiled on host per chunk to [128, KT, width] (flattened along the
    # free dim) so every chunk DMA is one contiguous run per partition
    xT = nc.declare_dram_parameter("xT", [128, KT * BT], mmdt, isOutput=False)
    # xw2[p, t] = x_weights[p mod 64, t]; Wbc/b0 are the xw-row of W_cell and
    # gate bias folded to the gate-psum quadrant layout (per-partition)
    xw2 = nc.declare_dram_parameter("xw2", [128, T], f32, isOutput=False)
    Wbc = nc.declare_dram_parameter("Wbc", [128, 2 * H], mmdt, isOutput=False)
    bq = nc.declare_dram_parameter("bq", [128, 2], f32, isOutput=False)
    Wd = nc.declare_dram_parameter("Wd", [128, KT * 128], mmdt, isOutput=False)
    Wxa = nc.declare_dram_parameter("Wxa", [128, G4], mmdt, isOutput=False)
    # bank-major: col = bank*4096 + k*1024 + half*512 + j
    Wh = nc.declare_dram_parameter("Wh", [128, 4 * G4], mmdt, isOutput=False)
    Wo = nc.declare_dram_parameter("Wo", [128, H], mmdt, isOutput=False)
    bout = nc.declare_dram_parameter("bout", [128, 1], f32, isOutput=False)
    ident = nc.declare_dram_parameter("ident", [128, BC], mmdt, isOutput=False)
    pred = nc.declare_dram_parameter("pred", [BC, 1], f32, isOutput=True)

    # flat free-dim offset of each chunk in xT
    choff = [0]
    for _, w in CHUNKS:
        choff.append(choff[-1] + KT * w)

    with ExitStack() as ctx:
        tc = ctx.enter_context(tile.TileContext(nc))
        singles = ctx.enter_context(tc.tile_pool(name="singles", bufs=1))
        xch = ctx.enter_context(tc.tile_pool(name="xch", bufs=3))
        work = ctx.enter_context(tc.tile_pool(name="work", bufs=2))
        state = ctx.enter_context(tc.tile_pool(name="state", bufs=2))
        dpp = ctx.enter_context(tc.tile_pool(name="dpp", bufs=2, space="PSUM"))
        gp = ctx.enter_context(tc.tile_pool(name="gp", bufs=4, space="PSUM"))
        tp = ctx.enter_context(tc.tile_pool(name="tp", bufs=2, space="PSUM"))

        # ---- small tensors first on the scalar ring (seed(0) needs them) ----
        xw2_sb = singles.tile([128, T], f32)
        nc.scalar.dma_start(xw2_sb[:], xw2[:])
        Wbc_sb = singles.tile([128, 2 * H], mmdt)
        nc.scalar.dma_start(Wbc_sb[:], Wbc[:])
        bq_sb = singles.tile([128, 2], f32)
        nc.scalar.dma_start(bq_sb[:], bq[:])

        # dummy activation so the sigmoid-set ACT_TABLE_LOAD (~1.3 us) runs
        # here, hidden under the DMA-paced head
        warmup = singles.tile([1, 1], f32)
        nc.vector.memset(warmup[:], 0.0)
        nc.scalar.activation(warmup[:], warmup[:], AF.Sigmoid)

        # ---- big loads ----
        # Everything sigma0/sigma1-critical rides the SYNC ring in strict
        # priority order (Wd -> ch0 -> Wxa -> Wh) so x chunks can't starve
        # the weights; later chunks follow. The scalar ring only carries the
        # small tensors, keeping the ACT sequencer free of DMA work.
        cht = [None] * NCH
        # k-piece boundaries per chunk (also the SCHED emission splits)
        KSPLITS = {0: [0, 40, KT], 1: [0, 40, KT], 2: [0, 40, KT],
                   3: [0, 26, 52, KT], 4: [0, 26, 52, KT], 5: [0, 26, 52, KT]}

        def chunk_dma(c, piece=None):
            w = CHUNKS[c][1]
            if cht[c] is None:
                cht[c] = xch.tile([128, KT * 192], mmdt, tag="xch",
                                  name=f"xch{c}")
            t_ = cht[c]
            ks = KSPLITS[c]
            pieces = list(zip(ks[:-1], ks[1:]))
            if piece is not None:
                pieces = [pieces[piece]]
            for k0, k1 in pieces:
                nc.sync.dma_start(t_[:, k0 * w:k1 * w],
                                  xT[:, choff[c] + k0 * w:choff[c] + k1 * w])

        Wd_sb = singles.tile([128, KT * 128], mmdt)
        nc.sync.dma_start(Wd_sb[:, :KSPLIT * 128], Wd[:, :KSPLIT * 128])
        chunk_dma(0, piece=0)
        nc.sync.dma_start(Wd_sb[:, KSPLIT * 128:], Wd[:, KSPLIT * 128:])
        chunk_dma(0, piece=1)
        Wxa_sb = singles.tile([128, G4], mmdt)
        nc.sync.dma_start(Wxa_sb[:], Wxa[:])
        Wh_sb = singles.tile([128, 4 * G4], mmdt)
        nc.sync.dma_start(Wh_sb[:, :2 * G4], Wh[:, :2 * G4])         # bank 0
        chunk_dma(1, piece=0)
        nc.sync.dma_start(Wh_sb[:, 2 * G4:], Wh[:, 2 * G4:])         # bank 1
        chunk_dma(1, piece=1)
        # chunk 2 claims the third pool buffer now; chunks 3+ are emitted
        # lazily after chunk c-3's last dp matmul (the pool WAR dependency
        # is trace-order-based, so the DMA must be emitted after the reads
        # it might overwrite)
        chunk_dma(2)

        id_sb = singles.tile([128, BC], mmdt)
        nc.scalar.dma_start(id_sb[:], ident[:])
        Wo_sb = singles.tile([128, H], mmdt)
        nc.scalar.dma_start(Wo_sb[:], Wo[:])
        bout_sb = singles.tile([128, 1], f32)
        nc.scalar.dma_start(bout_sb[:], bout[:])

        # quadrant (bank, half) of permuted gates = cols half*1024+bank*512
        def rh_quad(rh, bank, half):
            return rh[:, half * 2 * H + bank * H: half * 2 * H + (bank + 1) * H]

        # ---- down-projection machinery ----
        xaT_sb = [singles.tile([128, w], mmdt, name=f"xaT{c}")
                  for c, (_, w) in enumerate(CHUNKS)]

        def dp_mms(c, ps, k0, k1):
            w = CHUNKS[c][1]
            t_ = cht[c]
            for k in range(k0, k1):
                nc.tensor.matmul(ps[:], Wd_sb[:, k * 128:(k + 1) * 128],
                                 t_[:, k * w:(k + 1) * w],
                                 start=(k == 0), stop=(k == KT - 1))

        def chunk_copy(c, ps):
            # PSUM -> SBUF on the DVE (gpsimd cannot access PSUM); emitted in
            # post_chain so it sits behind the step's chain ops in the FIFO
            nc.vector.tensor_copy(xaT_sb[c][:], ps[:])

        # chunk 0: full dp in the head (DMA-paced)
        cur_ps = [None] * NCH
        cur_ps[0] = dpp.tile([128, 512], mybir.dt.float32, tag="dp",
                             name="dps0")[:, :CHUNKS[0][1]]
        dp_mms(0, cur_ps[0], 0, KT)
        chunk_dma(3)
        chunk_copy(0, cur_ps[0])

        # ---- LSTM ----
        # All chain tensors (incl. the cell state c) are 16-bit so every DVE
        # op runs in 2x packed mode; fp16's 10 mantissa bits keep the c
        # accumulation error ~2^-11 per step.
        hT_prev = None
        U_cur = work.tile([128, H], mmdt, tag="u")
        nc.vector.memset(U_cur[64:128, :], 0.0)
        g_next = None

        def seed(t):
            """Allocate the psum pair for step t and seed it with the xw/bias
            gate contribution on the ACT engine (out = Wbc*xw[t] + b, a rank-1
            update the PE would otherwise burn ~0.9 us/step on). Emitted at
            the TOP of the previous lstm_step so the seeds run in the ACT's
            idle window there (the 4-deep wait queue otherwise lets them
            slip between sigma_a and sigma_b, delaying the chain)."""
            g0 = gp.tile([128, H], mybir.dt.float32, tag="g", name="g0")
            g1 = gp.tile([128, H], mybir.dt.float32, tag="g", name="g1")
            nc.scalar.activation(g0[:], Wbc_sb[:, 0:H], AF.Identity,
                                 bias=bq_sb[:, 0:1], scale=xw2_sb[:, t:t + 1])
            nc.scalar.activation(g1[:], Wbc_sb[:, H:2 * H], AF.Identity,
                                 bias=bq_sb[:, 1:2], scale=xw2_sb[:, t:t + 1])
            return (g0, g1)

        def xa_mms(t, gpair, close=False):
            """Accumulate step t's xa gate contributions onto its seeded
            psum pair (start=False)."""
            col = t * BC
            ci = next(i for i, (o, w) in enumerate(CHUNKS) if o <= col < o + w)
            coff = col - CHUNKS[ci][0]
            lh = xaT_sb[ci][:, coff:coff + BC]
            for bank, g in ((0, gpair[0]), (1, gpair[1])):
                for half in range(2):
                    # skip_group_check: the sim's psum zero-region bookkeeping
                    # misaligns for partition-offset quadrants (HW zeroes only
                    # the written bytes); the seed supplies the initial value
                    nc.tensor.matmul(g[half * 64:(half + 1) * 64, :], lh,
                                     rh_quad(Wxa_sb, bank, half),
                                     start=False, stop=close,
                                     skip_group_check=True)

        def lstm_step(t, pe_fill=None, post_chain=None, pe_fill_post=None):
            nonlocal hT_prev, U_cur, g_next
            g0, g1 = g_next
            Sa = work.tile([128, H], mmdt, tag="sa")
            Sb = work.tile([128, H], mmdt, tag="sb")

            def acts_a():
                nc.scalar.activation(Sa[:], g0[:], AF.Sigmoid)

            if hT_prev is not None:
                for bank, g in ((0, g0), (1, g1)):
                    for k in range(4):
                        lh = hT_prev[:, k, :]
                        sp = k == 3
                        for half in range(2):
                            rhap = Wh_sb[:, bank * 4 * G4 // 2 + k * 2 * H
                                         + half * H: bank * 4 * G4 // 2
                                         + k * 2 * H + (half + 1) * H]
                            nc.tensor.matmul(g[half * 64:(half + 1) * 64, :],
                                             lh, rhap, start=False, stop=sp,
                                             skip_group_check=True)
                    if bank == 0:
                        acts_a()
                nc.scalar.activation(Sb[:], g1[:], AF.Sigmoid)
            else:
                acts_a()
                nc.scalar.activation(Sb[:], g1[:], AF.Sigmoid)

            # dp slab matmuls in the activation-chain shadow
            if pe_fill is not None:
                pe_fill()

            # Gate layout after the host fold (tanh(x) = 2*sig(2x)-1 with
            # c-gate columns pre-scaled by 2; forget bias +1 folded in):
            #   Sa[0:64] = sig(2*c~)   Sa[64:128] = sig(f+1)
            #   Sb[0:64] = sig(i)      Sb[64:128] = sig(o)
            #   m1 (psum, p64:128) = sig(f+1)*c_prev
            #   m2 (sbuf, p0:64)   = (sig(2c~)-0.5)*sig(i) = tanh(c~)*sig(i)/2
            # m1 lives in PSUM (SBUF operands of one DVE op must share a base
            # partition, and m2 sits at partitions 0:64 while m1 is at
            # 64:128) and must be fp32 (the DVE cannot write 16-bit PSUM)
            m1 = tp.tile([128, H], mybir.dt.float32, tag="tp", name="m1")
            nc.vector.tensor_mul(m1[64:128, :], Sa[64:128, :], U_cur[64:128, :])
            HH = H // 2
            m2 = work.tile([BC, H], mmdt, tag="m2")
            for x0 in (0, HH):
                nc.vector.scalar_tensor_tensor(m2[:, x0:x0 + HH], Sa[0:64, x0:x0 + HH],
                                               0.5, Sb[0:64, x0:x0 + HH],
                                               ALU.subtract, ALU.mult)
            U_nxt = work.tile([128, H], mmdt, tag="u")
            scn = work.tile([128, H], mmdt, tag="scn")
            for x0 in (0, HH):
                nc.vector.scalar_tensor_tensor(U_nxt[64:128, x0:x0 + HH], m2[:, x0:x0 + HH],
                                               2.0, m1[64:128, x0:x0 + HH],
                                               ALU.mult, ALU.add)
                nc.scalar.activation(scn[64:128, x0:x0 + HH], U_nxt[64:128, x0:x0 + HH],
                                     AF.Tanh)

            h = work.tile([128, H], mmdt, tag="h")
            hT = state.tile([128, 4, BC], mmdt, tag="hT")
            tps = tp.tile([128, 4 * BC], mmdt, tag="tp", name="tps")
            for half in range(2):
                nc.vector.tensor_mul(h[64:128, half * HH:(half + 1) * HH],
                                     scn[64:128, half * HH:(half + 1) * HH],
                                     Sb[64:128, half * HH:(half + 1) * HH])
                if t + 1 >= T:
                    continue          # pred reads h directly, not hT
                for k in (2 * half, 2 * half + 1):
                    nc.tensor.transpose(tps[:, k * BC:(k + 1) * BC],
                                        h[64:128, k * 128:(k + 1) * 128], id_sb[64:128, :])
                for k in (2 * half, 2 * half + 1):
                    nc.vector.tensor_copy(hT[:, k, :], tps[:, k * BC:(k + 1) * BC])
            # chunk-completion copies, then the next step's seed + xa
            # prefetch (which reads the copied tile)
            if post_chain is not None:
                post_chain()
            if t + 1 < T:
                g_next = seed(t + 1)
                xa_mms(t + 1, g_next)
            if pe_fill_post is not None:
                pe_fill_post()
            hT_prev = hT
            U_cur = U_nxt
            return h

        g_next = seed(0)
        xa_mms(0, g_next, close=True)
        pending_copies = []

        # chunk c>=1: matmuls spread over the steps before its first use
        # (k-ranges match the chunk's DMA pieces so each range only waits on
        # its own piece), copy at the end of the last emission step
        SCHED = {}
        _WINDOWS = {1: [0, 1], 2: [2, 3], 3: [4, 5, 6], 4: [7, 8, 9],
                    5: [10, 11, 12]}
        for c, steps in _WINDOWS.items():
            ks = KSPLITS[c]
            for s, k0, k1 in zip(steps, ks[:-1], ks[1:]):
                SCHED.setdefault(s, []).append((c, k0, k1))
        filled = {}

        def dp_fill(t):
            n = 0
            for c, k0, k1 in SCHED.get(t, []):
                if k0 == 0:
                    cur_ps[c] = dpp.tile([128, 512], mybir.dt.float32,
                                         tag="dp", name=f"dps{c}")[:, :CHUNKS[c][1]]
                dp_mms(c, cur_ps[c], k0, k1)
                if k1 == KT:
                    if c + 3 < NCH:
                        chunk_dma(c + 3)
                    pending_copies.append(c)
                n += 1
            filled[t] = n

        def tail_fill():
            while pending_copies:
                c = pending_copies.pop(0)
                chunk_copy(c, cur_ps[c])

        def make_warm(t):
            def warm_fill():
                if filled.get(t, 0) == 0 and t + 1 < T and hT_prev is not None:
                    warm = dpp.tile([128, 512], mybir.dt.float32, tag="dp", name="warm")
                    NW = 9
                    for w in range(NW):
                        nc.tensor.matmul(warm[0:64, :], hT_prev[:, w % 4, :],
                                         Wh_sb[:, 0:512], start=(w == 0), stop=(w == NW - 1))
            return warm_fill

        for t in range(T):
            h_last = lstm_step(t, pe_fill=(lambda t=t: dp_fill(t)),
                               post_chain=tail_fill, pe_fill_post=make_warm(t))

        # output head: pred = reduce(h * W_out_row) + b_out on the DVE --
        # no hT transposes/copies or PE matmuls on the tail
        scr = singles.tile([128, H], mmdt)
        red = singles.tile([128, 1], mybir.dt.float32)
        out_sb = singles.tile([128, 1], mybir.dt.float32)
        nc.vector.tensor_mul(scr[64:128, :], h_last[64:128, :], Wo_sb[64:128, :])
        nc.vector.tensor_reduce(red[64:128, :], scr[64:128, :],
                                mybir.AxisListType.X, ALU.add)
        nc.vector.tensor_add(out_sb[64:128, :], red[64:128, :], bout_sb[64:128, :])
        nc.sync.dma_start(pred[:], out_sb[64:128, :])

    nc.finalize()
    return nc


def _get_module():
    key = MM_DTYPE
    if key not in _CACHE:
        _CACHE[key] = _build_module()
    return _CACHE[key]


def _prep_inputs(x, W_down, W_cell, b_cell, W_out, b_out):
    mmnp = _np_mm_dtype()
    x = np.asarray(x, dtype=np.float32)
    W_down = np.asarray(W_down, dtype=np.float32)
    W_cell = np.asarray(W_cell, dtype=np.float32)
    b_cell = np.asarray(b_cell, dtype=np.float32)
    W_out = np.asarray(W_out, dtype=np.float32)
    b_out = np.asarray(b_out, dtype=np.float32)

    # shared tensors
    Wd_pad = np.zeros((NPAD, HIN), dtype=np.float32)
    Wd_pad[:N] = W_down
    # [NPAD, 128] -> per-k-tile layout [128, KT*128] (col block k = k-tile)
    Wd_host = np.ascontiguousarray(
        Wd_pad.reshape(KT, 128, HIN).transpose(1, 0, 2).reshape(128, KT * HIN)
    ).astype(mmnp)
    # Fold the LSTM's fixed affine pieces into the weights so the device can
    # use a single sigmoid for the gate banks (tanh(x) = 2*sig(2x) - 1 for
    # the c~ gate; c_new's tanh uses the ACT Tanh function directly):
    #   - c-gate columns (512:1024) scaled by 2  -> sig computes sig(2*c~)
    #   - forget bias +1 folded into b_cell
    Wmod = W_cell.astype(np.float64).copy()
    b_mod = b_cell.astype(np.float64).copy()
    b_mod[1024:1536] += 1.0
    Wmod[:, 512:1024] *= 2.0
    b_mod[512:1024] *= 2.0
    # permute gate columns [i|c|f|o] -> [c~|i|f|o]: the device wants the
    # early-needed gates (c~, f) in PSUM bank 0 of each partition half
    perm = np.concatenate([np.arange(512, 1024), np.arange(0, 512),
                           np.arange(1024, 1536), np.arange(1536, 2048)])
    Wmod = Wmod[:, perm]
    b_mod = b_mod[perm]
    Wxa_host = np.ascontiguousarray(Wmod[0:HIN]).astype(mmnp)            # [128, 2048]
    # xw-row of W_cell and gate bias in gate-psum quadrant layout:
    # quadrant (bank, half) = permuted cols half*1024+bank*512; tile col j of
    # bank b holds w[quad(b, p<64 ? 0 : 1)][j] on partition p
    w_row = Wmod[HIN]                                                    # [2048]
    Wbc_host = np.empty((128, 2 * H), dtype=np.float64)
    bq_host = np.empty((128, 2), dtype=np.float64)
    for bank in range(2):
        for half in range(2):
            quad = slice(half * 1024 + bank * 512, half * 1024 + (bank + 1) * 512)
            Wbc_host[half * 64:(half + 1) * 64, bank * H:(bank + 1) * H] = w_row[quad]
            bvals = b_mod[quad]
            assert np.ptp(bvals) == 0.0, "gate bias must be quadrant-constant"
            bq_host[half * 64:(half + 1) * 64, bank] = bvals[0]
    Wbc_host = Wbc_host.astype(mmnp)
    bq_host = bq_host.astype(np.float32)
    # W_h bank-major: col = bank*4096 + k*1024 + half*512 + j
    Whm = Wmod[HIN + 1:]                                                 # [512, 2048]
    Whb = np.zeros((128, 4 * G4), dtype=np.float64)
    for bank in range(2):
        for k in range(4):
            for half in range(2):
                src = Whm[128 * k:128 * (k + 1),
                          half * 1024 + bank * 512: half * 1024 + (bank + 1) * 512]
                Whb[:, bank * 4096 + k * 1024 + half * 512:
                    bank * 4096 + k * 1024 + (half + 1) * 512] = src
    Wh_host = np.ascontiguousarray(Whb).astype(mmnp)                     # [128, 8192]
    Wo_host = np.ascontiguousarray(
        np.broadcast_to(W_out.reshape(1, H), (128, H))).astype(mmnp)     # [128, 512]
    bout_host = np.full((128, 1), float(b_out[0]), dtype=np.float32)
    id_host = np.concatenate([np.eye(BC, dtype=np.float32)] * 2, axis=0).astype(mmnp)

    in_maps = []
    for i in range(NCORES):
        xs = x[i * BC:(i + 1) * BC]                       # [64, 16, 10001]
        # xT: [NPAD, 1024], column index = t*64 + b (t-major)
        xT_host = np.zeros((NPAD, BT), dtype=mmnp)
        xT_host[:N] = xs[:, :, :N].transpose(2, 1, 0).reshape(N, BT).astype(mmnp)
        # re-tile per chunk to [128, KT, width] flattened on the free dim so
        # each (chunk, k-range) transfer is contiguous per partition:
        # chunk block[p, k, j] = xT[k*128 + p, off + j]
        parts = []
        for off, w in CHUNKS:
            blk = xT_host[:, off:off + w].reshape(KT, 128, w).transpose(1, 0, 2)
            parts.append(blk.reshape(128, KT * w))
        xT_host = np.ascontiguousarray(np.concatenate(parts, axis=1))
        xw2_host = np.concatenate([xs[:, :, N]] * 2, axis=0).astype(np.float32)
        in_maps.append({
            "xT": xT_host,
            "xw2": xw2_host,
            "Wbc": Wbc_host,
            "bq": bq_host,
            "Wd": Wd_host,
            "Wxa": Wxa_host,
            "Wh": Wh_host,
            "Wo": Wo_host,
            "bout": bout_host,
            "ident": id_host,
        })
    return in_maps


def run(trace=False, **inputs):
    from concourse.bass_utils import run_bass_kernel_spmd

    nc = _get_module()
    in_maps = _prep_inputs(**inputs)
    res = run_bass_kernel_spmd(nc, in_maps, list(range(NCORES)), trace=trace)
    pred = np.concatenate([res.results[i]["pred"] for i in range(NCORES)], axis=0)
    return pred.astype(np.float32), res


def kernel(**inputs):
    pred, _ = run(trace=False, **inputs)
    return pred
